# revision 2
# baseline (speedup 1.0000x reference)
"""Single-head causal attention on 8 TRN2 NeuronCores (Bass/Tile).

Problem: x[B=8,T=4096,C=1024] @ {Wq,Wk,Wv}[C,HS=64] -> causal softmax
attention -> out[B,T,HS].

Sharding: data-parallel over batch — core b computes batch element b with
replicated projection weights (per the sharding hint).

Per-core dataflow (matmul operands bf16, fp32 PSUM accumulation):
  - x arrives host-side transposed, bf16-cast, AND permuted to a
    [partition, block, chunk, col] layout so every DMA is one fat 8KB
    descriptor per partition.  All bf16 constants ride ONE packed DMA.
  - x chunks ride the SCALAR HWDGE ring, issued UP FRONT (chunks 0-3 at
    the top, 4-7 threaded between the first pair ACTs) so the stream is
    never throttled by compute progress; the latency-critical small
    transfers (qT dup, kT interleave, v fold shift, v transposes) keep
    the sync ring to themselves.  (v1 issued chunk j+2 from inside
    proj(j) on the sync ring: every chunk landed barely-in-time and the
    PE stalled ~12us across the head + block boundaries.)
  - [qT;kT] = [Wq|Wk]^T @ xT per 512-wide query block (PSUM-accumulated
    over 8 c-chunks); vT row-packed across partition halves (even/odd
    c-chunks run col-group-CONCURRENT on the PE), folded, then moved to
    natural v[s,64] by DMA XBAR TRANSPOSES on the sync ring — the PE
    transposes of v1 (8.8us of PE time) are gone.  A ones column is
    appended so the PV matmul also produces softmax row-sums for free.
  - Scores are computed transposed (weiT[s,t] = kT^T @ qT) as row-packed
    pairs: two K=64 matmuls on disjoint PE row groups run concurrently.
  - exp is SPLIT between ScalarE (ACT out of PSUM, 1/sqrt(C) scale
    folded in; no running-max needed, logits are small) and the DVE
    (Schraudolph exp2 bit-trick: bitcast_bf16(int16(x*A+B)), ~1.8% rms,
    softmax normalization cancels most of it) for alternate off-diagonal
    pairs from block J0_DVE on — mid-kernel the two exp streams run in
    parallel and the kernel is PE-paced instead of ACT-paced.  Causal
    masks for diagonal tiles multiply on GpSimd (SBUF-only op) to keep
    the DVE budget for exps + evacuations.
  - Scheduling: projections for block j+2 are emitted as generators of
    small PE chunks, advanced TWO steps per pair iteration, with no
    forced drain at block boundaries (v1's drain_projgens() bunched
    whole projection blocks into multi-us PE bursts that starved
    ScalarE 2-7us at every boundary).  Deadlines are enforced lazily:
    qk(j+1) before priming the next block's score pairs, v(j) before
    block j's diagonal pairs.
  - The kernel stores outT[65, T] (values + sums row); the tiny
    normalization + transpose epilogue runs on the host.

Walrus --enable-ldw-opt is NOT usable: it rejects every bass-emitted
InstLdweights on this compiler build (verified by bisection down to a
single plain matmul), so LDWEIGHTS stay serialized with their matmuls.
"""

import numpy as np

import concourse.bacc as bacc
import concourse.bass as bass
import concourse.mybir as mybir
import concourse.tile as tile
from concourse import bass_utils

B, T, C, HS = 8, 4096, 1024, 64
TB = 512                 # query-block width (PSUM bank = 512 fp32)
NJ = T // TB             # 8 query blocks
NK = C // 128            # 8 contraction chunks
NS = T // 128            # 32 key tiles
SCALE = C ** -0.5

F32 = mybir.dt.float32
BF16 = mybir.dt.bfloat16
I16 = mybir.dt.int16
EXP = mybir.ActivationFunctionType.Exp
# DVE-side exp:
#   exp(SCALE*x) ~= bitcast_bf16(int16(x*SCH_A + SCH_B))
# (Schraudolph exp2 bit-trick at bf16 precision, ~1.8% rms on the
# offloaded scores; softmax normalization cancels most of it)
SCH_A = 128.0 * 1.4426950408889634 * SCALE
SCH_B = 127.0 * 128.0 - 5.59
# First block whose off-diagonal pairs alternate onto the DVE exp path.
# ScalarE carries ~50 of 72 pair-exps, DVE ~22; both fit under the PE
# wall with this split.
J0_DVE = 4


def build_program():
    nc = bacc.Bacc("TRN2", target_bir_lowering=False, debug=False)

    # x host-permuted to [p, j, k, t]: each chunk DMA is 128 fat 8KB
    # descriptors
    xT = nc.dram_tensor("xT", [128, NJ * NK * TB], BF16, kind="ExternalInput")
    # bf16 constants host-packed into one contiguous-per-partition tensor:
    # [wqk(8x128) | wv(8x64) | idenb(128, unused) | mask(128)] = 1792 cols.
    consts = nc.dram_tensor("consts", [128, 1792], BF16, kind="ExternalInput")
    # transposed output + softmax-sum row; the host epilogue divides and
    # transposes (0.06% of the FLOPs)
    out = nc.dram_tensor("out", [HS + 1, T], F32, kind="ExternalOutput")

    with tile.TileContext(nc) as tc:
        with (
            tc.tile_pool(name="const", bufs=1) as constp,
            tc.tile_pool(name="persist", bufs=1) as persist,
            tc.tile_pool(name="stage", bufs=3) as stg,
            tc.tile_pool(name="expp", bufs=4) as expp,
            tc.tile_pool(name="fin", bufs=8) as finp,
            tc.tile_pool(name="ps_pp", bufs=2, space=bass.MemorySpace.PSUM) as ps_pp,
            tc.tile_pool(name="ps_wei", bufs=2, space=bass.MemorySpace.PSUM) as ps_wei,
            tc.tile_pool(name="ps_out", bufs=2, space=bass.MemorySpace.PSUM) as ps_out,
        ):
            consts_sb = constp.tile([128, 1792], BF16)
            nc.sync.dma_start(consts_sb[:], consts[:])
            wqk_sb = consts_sb[:, 0:1024].rearrange("p (k m) -> p k m", m=128)
            wv_sb = consts_sb[:, 1024:1536].rearrange("p (k m) -> p k m", m=HS)
            mask_sb = consts_sb[:, 1664:1792]

            # whole xT resident in SBUF (bf16, 64KB/partition); chunks 0-3
            # stream up front on the scalar HWDGE ring (chunk 0 as two
            # half-chunks so block 0's projection starts earliest), 4-7
            # threaded between the first pair ACTs below.
            xt = persist.tile([128, NJ, NK, TB], BF16)

            def load_x(jj):
                nc.scalar.dma_start(
                    xt[:, jj, :, :].rearrange("p k t -> p (k t)"),
                    xT[:, jj * NK * TB : (jj + 1) * NK * TB],
                )

            nc.scalar.dma_start(
                xt[:, 0, 0:4, :].rearrange("p k t -> p (k t)"),
                xT[:, 0 : 4 * TB],
            )
            nc.scalar.dma_start(
                xt[:, 0, 4:8, :].rearrange("p k t -> p (k t)"),
                xT[:, 4 * TB : 8 * TB],
            )
            load_x(1)
            load_x(2)
            load_x(3)
            pending_x = [4, 5, 6, 7]

            # tiny dummy exp: pulls ACT_TABLE_LOAD (~2.7us) into the head
            warm = finp.tile([1, 1], F32, tag="warm", bufs=1)
            nc.scalar.activation(warm[:], consts_sb[0:1, 0:1], EXP, scale=SCALE)

            # keys, transposed + interleaved: pair p holds kT of s-tile 2p on
            # partitions 0-63 and of s-tile 2p+1 on partitions 64-127
            kTI = persist.tile([128, (NS // 2) * 128], BF16)
            # values + ones col, padded to 80 so each s-tile's row offset is
            # 32B-aligned (required by the xbar DMA-transpose destination)
            v_all = persist.tile([128, NS, 80], BF16)
            nc.vector.memset(v_all[:, :, HS : HS + 1], 1.0)

            # Projections are emitted as GENERATORS of small PE chunks,
            # advanced two steps per pair iteration: no multi-us proj burst
            # ever sits in the in-order PE queue between ScalarE's next
            # ACT and its wei producer, and the PE stream stays dense.
            qk_st = {}

            def proj_qk(j):
                # [qT;kT] projection: rows 0-63 = qT, rows 64-127 = kT
                qk_ps = ps_pp.tile([128, TB], F32, tag="pp", name="qk_ps")
                for k in range(NK):
                    nc.tensor.matmul(
                        qk_ps[:], wqk_sb[:, k, :], xt[:, j, k, :],
                        start=(k == 0), stop=(k == NK - 1),
                    )
                    if k % 2 == 1:
                        yield
                qkt = stg.tile([128, TB], BF16, tag="qkt", name="qkt")
                nc.vector.tensor_copy(qkt[:], qk_ps[:])
                # duplicate qT onto partitions 64-127 (row-packed QK rhs)
                qt2 = stg.tile([128, TB], BF16, tag="qt2", name="qt2")
                nc.sync.dma_start(qt2[64:128, :], qkt[0:64, :])
                # interleave this block's 4 kT tiles into the pair layout:
                # even tiles -> partitions 0-63, odd tiles -> 64-127
                kt_src = qkt[64:128, :].rearrange(
                    "p (a e b) -> p a e b", e=2, b=128
                )
                kt_dst = kTI[:, 256 * j : 256 * (j + 1)].rearrange(
                    "p (a b) -> p a b", b=128
                )
                nc.sync.dma_start(kt_dst[0:64, :, :], kt_src[:, :, 0, :])
                nc.sync.dma_start(kt_dst[64:128, :, :], kt_src[:, :, 1, :])
                qk_st[j] = (qkt, qt2)
                yield

            def proj_v(j):
                # v projection, row-packed across partition halves (even
                # c-chunks on partitions 0-63, odd on 64-127; the halves run
                # col-group-concurrent on the PE), folded, then DMA-XBAR
                # transposed to natural v[s,64]
                vt_ps = ps_pp.tile([128, TB], F32, tag="pp", name="vt_ps")
                for k in range(NK):
                    lo = HS * (k % 2)
                    nc.tensor.matmul(
                        vt_ps[lo : lo + HS, :], wv_sb[:, k, :],
                        xt[:, j, k, :],
                        start=(k <= 1), stop=(k >= NK - 2),
                        skip_group_check=True,
                    )
                    if k % 2 == 1:
                        yield
                vt_hi = stg.tile([128, TB], F32, tag="vt_hi", name="vt_hi")
                nc.vector.tensor_copy(vt_hi[64:128, :], vt_ps[64:128, :])
                vt_lo = stg.tile([HS, TB], F32, tag="vt_lo", name="vt_lo")
                nc.sync.dma_start(vt_lo[:], vt_hi[64:128, :])
                vt_sb = stg.tile([HS, TB], BF16, tag="vt_sb", name="vt_sb")
                nc.vector.tensor_add(vt_sb[:], vt_ps[0:HS, :], vt_lo[:])
                yield
                # natural-order v[s,64] via the DMA XBAR transpose engine
                # (sync ring): zero PE time, 32B-aligned dest offsets
                for rr in range(TB // 128):
                    nc.sync.dma_start_transpose(
                        v_all[:, 4 * j + rr, 0:HS],
                        vt_sb[:, rr * 128 : (rr + 1) * 128],
                    )
                    if rr % 2 == 1:
                        yield

            def issue_wei(j, qkt, qt2, p):
                iA, iB = 2 * p, 2 * p + 1
                rA, rB = iA - 4 * j, iB - 4 * j
                c0A = 128 * rA if rA > 0 else 0
                c0B = 128 * rB if rB > 0 else 0
                wei = ps_wei.tile([128, 2 * TB], F32, tag="wei", name="wei")
                nc.tensor.matmul(
                    wei[:, c0A:TB],
                    kTI[0:64, 128 * p : 128 * (p + 1)],
                    qkt[0:HS, c0A:TB],
                    start=True, stop=True,
                )
                nc.tensor.matmul(
                    wei[:, TB + c0B : 2 * TB],
                    kTI[64:128, 128 * p : 128 * (p + 1)],
                    qt2[64:128, c0B:TB],
                    start=True, stop=True,
                )
                return wei, c0A, c0B, rA, rB

            import itertools as _it

            projgens = []  # [block j, generator] in block order

            def advance_projgens(n=1):
                while n > 0 and projgens:
                    try:
                        next(projgens[0][1])
                        n -= 1
                    except StopIteration:
                        projgens.pop(0)

            def drain_projgens():
                while projgens:
                    advance_projgens(1)

            def ensure_qk(jj):
                # force-advance until block jj's qk projection (and its
                # qt2/kTI transfers) has been emitted
                while jj not in qk_st and projgens:
                    advance_projgens(1)

            def ensure_v(jj):
                # force-advance until block jj's v tiles are fully emitted
                while projgens and projgens[0][0] <= jj:
                    advance_projgens(1)

            # minimal prologue: block 0's projections are drained before
            # priming the first score pair (its PVs need v_all[0..3]), but
            # block 1's stream through the pair loop
            projgens.append([0, _it.chain(proj_qk(0), proj_v(0))])
            drain_projgens()
            projgens.append([1, _it.chain(proj_qk(1), proj_v(1))])
            # prime the wei pipeline (2 tiles = full psum ring)
            pend = [
                issue_wei(0, *qk_st[0], 0),
                issue_wei(0, *qk_st[0], 1),
            ]

            for j in range(NJ):
                t0 = j * TB
                qkt, qt2 = qk_st.pop(j)
                n_pairs = 2 * j + 2
                if j + 2 < NJ:
                    projgens.append(
                        [j + 2, _it.chain(proj_qk(j + 2), proj_v(j + 2))]
                    )

                outT_ps = ps_out.tile([HS + 1, TB], F32, tag="outT")
                for p in range(n_pairs):
                    if p == 2 * j:
                        # diagonal pairs read this block's fresh v tiles
                        ensure_v(j)
                    wei, c0A, c0B, rA, rB = pend.pop(0)
                    iA, iB = 2 * p, 2 * p + 1
                    if pending_x:
                        load_x(pending_x.pop(0))
                    # split the exp mass: alternate off-diagonal pairs ride
                    # the DVE exp2 trick from block J0_DVE on, so the two
                    # exp streams run in parallel mid-kernel
                    if j >= J0_DVE and rA < 0 and rB < 0 and p % 2 == 1:
                        exi = expp.tile([128, 2 * TB], I16, tag="exp")
                        nc.vector.tensor_scalar(
                            exi[:], wei[:], SCH_A, SCH_B,
                            mybir.AluOpType.mult, mybir.AluOpType.add,
                        )
                        ex = exi.bitcast(BF16)
                    else:
                        ex = expp.tile([128, 2 * TB], BF16, tag="exp")
                        # one ACT covers both halves; the dead gap holds
                        # bounded stale scores and is never read by PV
                        nc.scalar.activation(
                            ex[:, c0A : 2 * TB], wei[:, c0A : 2 * TB], EXP,
                            scale=SCALE,
                        )
                    # causal masks for diagonal tiles ride GpSimd (SBUF-only
                    # op) to keep the DVE budget for exps + evacuations
                    if rA >= 0:
                        nc.gpsimd.tensor_mul(
                            ex[:, c0A : c0A + 128], ex[:, c0A : c0A + 128],
                            mask_sb[:],
                        )
                    if rB >= 0:
                        nc.gpsimd.tensor_mul(
                            ex[:, TB + c0B : TB + c0B + 128],
                            ex[:, TB + c0B : TB + c0B + 128],
                            mask_sb[:],
                        )
                    if p + 2 < n_pairs:
                        pend.append(issue_wei(j, qkt, qt2, p + 2))
                    # two small slices of upcoming blocks' projections
                    advance_projgens(2)
                    # prime next block's first two score pairs BEFORE the
                    # last PV matmuls, so ScalarE rolls straight into the
                    # next block with no boundary stall
                    if p == n_pairs - 1 and j + 1 < NJ:
                        ensure_qk(j + 1)
                        pend = [
                            issue_wei(j + 1, *qk_st[j + 1], 0),
                            issue_wei(j + 1, *qk_st[j + 1], 1),
                        ]
                    nc.tensor.matmul(
                        outT_ps[:, c0A:TB],
                        v_all[:, iA, 0 : HS + 1],
                        ex[:, c0A:TB],
                        start=(p == 0), stop=False,
                        skip_group_check=True,
                    )
                    nc.tensor.matmul(
                        outT_ps[:, c0B:TB],
                        v_all[:, iB, 0 : HS + 1],
                        ex[:, TB + c0B : 2 * TB],
                        start=False, stop=(p == n_pairs - 1),
                        skip_group_check=True,
                    )

                # evacuate the transposed accumulator (values + sums row);
                # normalization happens in the host epilogue
                outT_sb = stg.tile([HS + 1, TB], F32, tag="outT_sb")
                nc.vector.tensor_copy(outT_sb[:], outT_ps[:])
                st_eng = nc.sync if j == NJ - 1 else nc.gpsimd
                st_eng.dma_start(out[:, t0 : t0 + TB], outT_sb[:])

            drain_projgens()

    nc.compile()
    return nc


_CACHE = {}


def _get_program():
    if "nc" not in _CACHE:
        _CACHE["nc"] = build_program()
    return _CACHE["nc"]


def _make_in_maps(inputs):
    import ml_dtypes

    x = np.asarray(inputs["x"], dtype=np.float32)
    Wq = np.asarray(inputs["Wq"], dtype=np.float32)
    Wk = np.asarray(inputs["Wk"], dtype=np.float32)
    Wv = np.asarray(inputs["Wv"], dtype=np.float32)
    wqk = np.concatenate([Wq, Wk], axis=1)  # [C, 128]
    consts = np.concatenate(
        [
            wqk.reshape(NK, 128, 128).transpose(1, 0, 2).reshape(128, 1024),
            Wv.reshape(NK, 128, HS).transpose(1, 0, 2).reshape(128, 512),
            np.eye(128, dtype=np.float32),
            np.triu(np.ones((128, 128), dtype=np.float32)),
        ],
        axis=1,
    ).astype(ml_dtypes.bfloat16)
    consts = np.ascontiguousarray(consts)
    in_maps = []
    for b in range(B):
        in_maps.append(
            {
                "xT": np.ascontiguousarray(
                    x[b].T.reshape(NK, 128, NJ, TB)
                    .transpose(1, 2, 0, 3)
                    .reshape(128, NJ * NK * TB)
                ).astype(ml_dtypes.bfloat16),
                "consts": consts,
            }
        )
    return in_maps


def kernel(x, Wk, Wq, Wv):
    nc = _get_program()
    in_maps = _make_in_maps({"x": x, "Wq": Wq, "Wk": Wk, "Wv": Wv})
    res = bass_utils.run_bass_kernel_spmd(nc, in_maps, core_ids=list(range(B)))
    outs = []
    for b in range(B):
        oT = res.results[b]["out"]  # [HS+1, T]: value rows + softmax sums
        outs.append(np.ascontiguousarray((oT[:HS] / oT[HS : HS + 1]).T))
    return np.stack(outs, axis=0).astype(np.float32)


# revision 3
# speedup vs baseline: 1.1203x; 1.1203x over previous
"""Single-head causal attention on 8 TRN2 NeuronCores (Bass/Tile).

Problem: x[B=8,T=4096,C=1024] @ {Wq,Wk,Wv}[C,HS=64] -> causal softmax
attention -> out[B,T,HS].

Sharding: data-parallel over batch — core b computes batch element b with
replicated projection weights (per the sharding hint).

Per-core dataflow (matmul operands bf16, fp32 PSUM accumulation):
  - x arrives host-side transposed, bf16-cast, AND permuted to a
    [partition, block, chunk, col] layout so every DMA is one fat 8KB
    descriptor per partition.  All bf16 constants ride ONE packed DMA.
  - x chunks are PREFETCHED: chunk 0 rides the sync ring right behind
    the consts, chunks 1-3 issue up front on the scalar HWDGE ring, and
    4-7 thread between the first pair ACTs — the stream is never
    throttled by compute progress.  (v1 issued chunk j+2 from inside
    proj(j): every chunk landed barely-in-time and the PE stalled ~12us
    across the head + block boundaries, re-throttling HAM.)
  - [qT;kT] = [Wq|Wk]^T @ xT per 512-wide query block (PSUM-accumulated
    over 8 c-chunks); vT row-packed across partition halves (even/odd
    c-chunks run col-group-CONCURRENT on the PE) then folded and
    PE-transposed to natural v[s,64] with a ones column appended so the
    PV matmul also produces softmax row-sums for free.
  - kT is kept on BOTH partition halves (kT_all[p, block, tile, 128]):
    two fat [64,512] partition-copies per block replace v1's interleave
    DMAs (whose 256B descriptors took ~4us to land and stalled the first
    score matmuls at the head).  Scores are computed transposed
    (weiT[s,t] = kT^T @ qT) as row-packed pairs: two K=64 matmuls on
    disjoint PE row groups run concurrently — pair p takes tile 2p from
    kT_all[0:64] and tile 2p+1 from kT_all[64:128].
  - exp is SPLIT between ScalarE (ACT out of PSUM, 1/sqrt(C) scale
    folded in; no running-max needed, logits are small) and the DVE
    (Schraudolph exp2 bit-trick: bitcast_bf16(int16(x*A+B)), ~1.8% rms,
    softmax normalization cancels most of it) for alternate off-diagonal
    pairs from block J0_DVE on — mid-kernel the two exp streams run in
    parallel and the kernel is PE-paced instead of ACT-paced.  Causal
    masks for diagonal tiles multiply on GpSimd (SBUF-only op) to keep
    the DVE budget for exps + evacuations.
  - Scheduling: projections for block j+2 are emitted as generators of
    small PE chunks, advanced TWO steps per pair iteration, with no
    forced drain at block boundaries (v1's drain_projgens() bunched
    whole projection blocks into multi-us PE bursts that starved
    ScalarE 2-7us at every boundary).  Deadlines are enforced lazily:
    qk(j+1) before priming the next block's score pairs, v(j) before
    block j's diagonal pairs.
  - The kernel stores outT[65, T] (values + sums row); the tiny
    normalization + transpose epilogue runs on the host.

Rejected by measurement: DMA-XBAR transposes for v (1.2-1.6us each on
the in-order sync queue; head-of-line-blocked the next block's dup
transfers and starved ScalarE 9-24us).  Walrus --enable-ldw-opt (rejects
every bass-emitted InstLdweights on this compiler build).
"""

import numpy as np

import concourse.bacc as bacc
import concourse.bass as bass
import concourse.mybir as mybir
import concourse.tile as tile
from concourse import bass_utils

B, T, C, HS = 8, 4096, 1024, 64
TB = 512                 # query-block width (PSUM bank = 512 fp32)
NJ = T // TB             # 8 query blocks
NK = C // 128            # 8 contraction chunks
NS = T // 128            # 32 key tiles
SCALE = C ** -0.5

F32 = mybir.dt.float32
BF16 = mybir.dt.bfloat16
I16 = mybir.dt.int16
EXP = mybir.ActivationFunctionType.Exp
# DVE-side exp:
#   exp(SCALE*x) ~= bitcast_bf16(int16(x*SCH_A + SCH_B))
SCH_A = 128.0 * 1.4426950408889634 * SCALE
SCH_B = 127.0 * 128.0 - 5.59
# First block whose off-diagonal pairs alternate onto the DVE exp path.
J0_DVE = 4


def build_program():
    nc = bacc.Bacc("TRN2", target_bir_lowering=False, debug=False)

    # x host-permuted to [p, j, k, t]: each chunk DMA is 128 fat 8KB
    # descriptors
    xT = nc.dram_tensor("xT", [128, NJ * NK * TB], BF16, kind="ExternalInput")
    # bf16 constants host-packed into one contiguous-per-partition tensor:
    # [wqk(8x128) | wv(8x64) | idenb(128) | mask(128)] = 1792 cols.
    consts = nc.dram_tensor("consts", [128, 1792], BF16, kind="ExternalInput")
    # transposed output + softmax-sum row; the host epilogue divides and
    # transposes (0.06% of the FLOPs)
    out = nc.dram_tensor("out", [HS + 1, T], F32, kind="ExternalOutput")

    with tile.TileContext(nc) as tc:
        with (
            tc.tile_pool(name="const", bufs=1) as constp,
            tc.tile_pool(name="persist", bufs=1) as persist,
            tc.tile_pool(name="stage", bufs=3) as stg,
            tc.tile_pool(name="expp", bufs=4) as expp,
            tc.tile_pool(name="fin", bufs=8) as finp,
            tc.tile_pool(name="ps_pp", bufs=2, space=bass.MemorySpace.PSUM) as ps_pp,
            tc.tile_pool(name="ps_wei", bufs=2, space=bass.MemorySpace.PSUM) as ps_wei,
            tc.tile_pool(name="ps_out", bufs=2, space=bass.MemorySpace.PSUM) as ps_out,
        ):
            consts_sb = constp.tile([128, 1792], BF16)
            nc.sync.dma_start(consts_sb[:], consts[:])
            wqk_sb = consts_sb[:, 0:1024].rearrange("p (k m) -> p k m", m=128)
            wv_sb = consts_sb[:, 1024:1536].rearrange("p (k m) -> p k m", m=HS)
            idenb_sb = consts_sb[:, 1536:1664]
            mask_sb = consts_sb[:, 1664:1792]

            # whole xT resident in SBUF (bf16, 64KB/partition).  Chunk 0
            # rides the sync ring right behind consts (needed first);
            # chunks 1-3 up front on the scalar HWDGE ring; 4-7 threaded
            # between the first pair ACTs.
            xt = persist.tile([128, NJ, NK, TB], BF16)

            def load_x(jj, eng):
                eng.dma_start(
                    xt[:, jj, :, :].rearrange("p k t -> p (k t)"),
                    xT[:, jj * NK * TB : (jj + 1) * NK * TB],
                )

            nc.sync.dma_start(
                xt[:, 0, 0:4, :].rearrange("p k t -> p (k t)"),
                xT[:, 0 : 4 * TB],
            )
            nc.sync.dma_start(
                xt[:, 0, 4:8, :].rearrange("p k t -> p (k t)"),
                xT[:, 4 * TB : 8 * TB],
            )
            load_x(1, nc.scalar)
            load_x(2, nc.scalar)
            load_x(3, nc.scalar)
            pending_x = [4, 5, 6, 7]

            # tiny dummy exp: pulls ACT_TABLE_LOAD (~2.7us) into the head
            warm = finp.tile([1, 1], F32, tag="warm", bufs=1)
            nc.scalar.activation(warm[:], consts_sb[0:1, 0:1], EXP, scale=SCALE)

            # kT on BOTH partition halves: [p, block, tile r, 128].
            # Score pair p uses [0:64, p//2, 2*(p%2)] and
            # [64:128, p//2, 2*(p%2)+1].
            kT_all = persist.tile([128, NJ, 4, 128], BF16)
            # values + ones col, padded to 80 (32B-aligned tile offsets)
            v_all = persist.tile([128, NS, 80], BF16)
            nc.vector.memset(v_all[:, :, HS : HS + 1], 1.0)

            # Projections are emitted as GENERATORS of small PE chunks,
            # advanced two steps per pair iteration: no multi-us proj burst
            # ever sits in the in-order PE queue between ScalarE's next
            # ACT and its wei producer, and the PE stream stays dense.
            qk_st = {}

            def proj_qk(j):
                # [qT;kT] projection: rows 0-63 = qT, rows 64-127 = kT
                qk_ps = ps_pp.tile([128, TB], F32, tag="pp", name="qk_ps")
                for k in range(NK):
                    nc.tensor.matmul(
                        qk_ps[:], wqk_sb[:, k, :], xt[:, j, k, :],
                        start=(k == 0), stop=(k == NK - 1),
                    )
                    if k % 2 == 1:
                        yield
                qkt = stg.tile([128, TB], BF16, tag="qkt", name="qkt")
                nc.vector.tensor_copy(qkt[:], qk_ps[:])
                # duplicate qT onto partitions 64-127 (row-packed QK rhs)
                qt2 = stg.tile([128, TB], BF16, tag="qt2", name="qt2")
                nc.sync.dma_start(qt2[64:128, :], qkt[0:64, :])
                # kT to both halves of kT_all: two fat [64,512] copies
                kt_dst = kT_all[:, j, :, :].rearrange("p r b -> p (r b)")
                nc.sync.dma_start(kt_dst[0:64, :], qkt[64:128, :])
                nc.sync.dma_start(kt_dst[64:128, :], qkt[64:128, :])
                qk_st[j] = (qkt, qt2)
                yield

            def proj_v(j):
                # v projection, row-packed across partition halves (even
                # c-chunks on partitions 0-63, odd on 64-127; the halves
                # run col-group-concurrent on the PE), then folded and
                # PE-transposed to natural v[s,64]
                vt_ps = ps_pp.tile([128, TB], F32, tag="pp", name="vt_ps")
                for k in range(NK):
                    lo = HS * (k % 2)
                    nc.tensor.matmul(
                        vt_ps[lo : lo + HS, :], wv_sb[:, k, :],
                        xt[:, j, k, :],
                        start=(k <= 1), stop=(k >= NK - 2),
                        skip_group_check=True,
                    )
                    if k % 2 == 1:
                        yield
                vt_hi = stg.tile([128, TB], F32, tag="vt_hi", name="vt_hi")
                nc.vector.tensor_copy(vt_hi[64:128, :], vt_ps[64:128, :])
                vt_lo = stg.tile([HS, TB], F32, tag="vt_lo", name="vt_lo")
                nc.sync.dma_start(vt_lo[:], vt_hi[64:128, :])
                vt_sb = stg.tile([HS, TB], BF16, tag="vt_sb", name="vt_sb")
                nc.vector.tensor_add(vt_sb[:], vt_ps[0:HS, :], vt_lo[:])
                yield
                for rr in range(TB // 128):
                    vtr_ps = ps_pp.tile(
                        [128, HS], BF16, tag="pp", name="vtr_ps"
                    )
                    nc.tensor.transpose(
                        vtr_ps[:], vt_sb[:, rr * 128 : (rr + 1) * 128],
                        idenb_sb[:HS, :HS],
                    )
                    nc.vector.tensor_copy(
                        v_all[:, 4 * j + rr, 0:HS], vtr_ps[:]
                    )
                    if rr % 2 == 1:
                        yield

            def issue_wei(j, qkt, qt2, p):
                iA, iB = 2 * p, 2 * p + 1
                rA, rB = iA - 4 * j, iB - 4 * j
                c0A = 128 * rA if rA > 0 else 0
                c0B = 128 * rB if rB > 0 else 0
                wei = ps_wei.tile([128, 2 * TB], F32, tag="wei", name="wei")
                nc.tensor.matmul(
                    wei[:, c0A:TB],
                    kT_all[0:64, p // 2, 2 * (p % 2), :],
                    qkt[0:HS, c0A:TB],
                    start=True, stop=True,
                )
                nc.tensor.matmul(
                    wei[:, TB + c0B : 2 * TB],
                    kT_all[64:128, p // 2, 2 * (p % 2) + 1, :],
                    qt2[64:128, c0B:TB],
                    start=True, stop=True,
                )
                return wei, c0A, c0B, rA, rB

            import itertools as _it

            projgens = []  # [block j, generator] in block order

            def advance_projgens(n=1):
                while n > 0 and projgens:
                    try:
                        next(projgens[0][1])
                        n -= 1
                    except StopIteration:
                        projgens.pop(0)

            def drain_projgens():
                while projgens:
                    advance_projgens(1)

            def ensure_qk(jj):
                while jj not in qk_st and projgens:
                    advance_projgens(1)

            def ensure_v(jj):
                # force-advance until block jj's v tiles are fully emitted
                while projgens and projgens[0][0] <= jj:
                    advance_projgens(1)

            # minimal prologue: block 0's projections are drained before
            # priming the first score pair (its PVs need v_all[0..3]), but
            # block 1's stream through the pair loop
            projgens.append([0, _it.chain(proj_qk(0), proj_v(0))])
            drain_projgens()
            projgens.append([1, _it.chain(proj_qk(1), proj_v(1))])
            # prime the wei pipeline (2 tiles = full psum ring)
            pend = [
                issue_wei(0, *qk_st[0], 0),
                issue_wei(0, *qk_st[0], 1),
            ]

            for j in range(NJ):
                t0 = j * TB
                qkt, qt2 = qk_st.pop(j)
                n_pairs = 2 * j + 2
                if j + 2 < NJ:
                    projgens.append(
                        [j + 2, _it.chain(proj_qk(j + 2), proj_v(j + 2))]
                    )

                outT_ps = ps_out.tile([HS + 1, TB], F32, tag="outT")
                for p in range(n_pairs):
                    if p == 2 * j:
                        # diagonal pairs read this block's fresh v tiles
                        ensure_v(j)
                    wei, c0A, c0B, rA, rB = pend.pop(0)
                    iA, iB = 2 * p, 2 * p + 1
                    if pending_x:
                        load_x(pending_x.pop(0), nc.scalar)
                    # split the exp mass: alternate off-diagonal pairs ride
                    # the DVE exp2 trick from block J0_DVE on, so the two
                    # exp streams run in parallel mid-kernel
                    if j >= J0_DVE and rA < 0 and rB < 0 and p % 2 == 1:
                        exi = expp.tile([128, 2 * TB], I16, tag="exp")
                        nc.vector.tensor_scalar(
                            exi[:], wei[:], SCH_A, SCH_B,
                            mybir.AluOpType.mult, mybir.AluOpType.add,
                        )
                        ex = exi.bitcast(BF16)
                    else:
                        ex = expp.tile([128, 2 * TB], BF16, tag="exp")
                        # one ACT covers both halves; the dead gap holds
                        # bounded stale scores and is never read by PV
                        nc.scalar.activation(
                            ex[:, c0A : 2 * TB], wei[:, c0A : 2 * TB], EXP,
                            scale=SCALE,
                        )
                    # causal masks for diagonal tiles ride GpSimd (SBUF-only
                    # op) to keep the DVE budget for exps + evacuations
                    if rA >= 0:
                        nc.gpsimd.tensor_mul(
                            ex[:, c0A : c0A + 128], ex[:, c0A : c0A + 128],
                            mask_sb[:],
                        )
                    if rB >= 0:
                        nc.gpsimd.tensor_mul(
                            ex[:, TB + c0B : TB + c0B + 128],
                            ex[:, TB + c0B : TB + c0B + 128],
                            mask_sb[:],
                        )
                    if p + 2 < n_pairs:
                        pend.append(issue_wei(j, qkt, qt2, p + 2))
                    # two small slices of upcoming blocks' projections
                    advance_projgens(2)
                    # prime next block's first two score pairs BEFORE the
                    # last PV matmuls, so ScalarE rolls straight into the
                    # next block with no boundary stall
                    if p == n_pairs - 1 and j + 1 < NJ:
                        ensure_qk(j + 1)
                        pend = [
                            issue_wei(j + 1, *qk_st[j + 1], 0),
                            issue_wei(j + 1, *qk_st[j + 1], 1),
                        ]
                    nc.tensor.matmul(
                        outT_ps[:, c0A:TB],
                        v_all[:, iA, 0 : HS + 1],
                        ex[:, c0A:TB],
                        start=(p == 0), stop=False,
                        skip_group_check=True,
                    )
                    nc.tensor.matmul(
                        outT_ps[:, c0B:TB],
                        v_all[:, iB, 0 : HS + 1],
                        ex[:, TB + c0B : 2 * TB],
                        start=False, stop=(p == n_pairs - 1),
                        skip_group_check=True,
                    )

                # evacuate the transposed accumulator (values + sums row);
                # normalization happens in the host epilogue
                outT_sb = stg.tile([HS + 1, TB], F32, tag="outT_sb")
                nc.vector.tensor_copy(outT_sb[:], outT_ps[:])
                st_eng = nc.sync if j == NJ - 1 else nc.gpsimd
                st_eng.dma_start(out[:, t0 : t0 + TB], outT_sb[:])

            drain_projgens()

    nc.compile()
    return nc


_CACHE = {}


def _get_program():
    if "nc" not in _CACHE:
        _CACHE["nc"] = build_program()
    return _CACHE["nc"]


def _make_in_maps(inputs):
    import ml_dtypes

    x = np.asarray(inputs["x"], dtype=np.float32)
    Wq = np.asarray(inputs["Wq"], dtype=np.float32)
    Wk = np.asarray(inputs["Wk"], dtype=np.float32)
    Wv = np.asarray(inputs["Wv"], dtype=np.float32)
    wqk = np.concatenate([Wq, Wk], axis=1)  # [C, 128]
    consts = np.concatenate(
        [
            wqk.reshape(NK, 128, 128).transpose(1, 0, 2).reshape(128, 1024),
            Wv.reshape(NK, 128, HS).transpose(1, 0, 2).reshape(128, 512),
            np.eye(128, dtype=np.float32),
            np.triu(np.ones((128, 128), dtype=np.float32)),
        ],
        axis=1,
    ).astype(ml_dtypes.bfloat16)
    consts = np.ascontiguousarray(consts)
    in_maps = []
    for b in range(B):
        in_maps.append(
            {
                "xT": np.ascontiguousarray(
                    x[b].T.reshape(NK, 128, NJ, TB)
                    .transpose(1, 2, 0, 3)
                    .reshape(128, NJ * NK * TB)
                ).astype(ml_dtypes.bfloat16),
                "consts": consts,
            }
        )
    return in_maps


def kernel(x, Wk, Wq, Wv):
    nc = _get_program()
    in_maps = _make_in_maps({"x": x, "Wq": Wq, "Wk": Wk, "Wv": Wv})
    res = bass_utils.run_bass_kernel_spmd(nc, in_maps, core_ids=list(range(B)))
    outs = []
    for b in range(B):
        oT = res.results[b]["out"]  # [HS+1, T]: value rows + softmax sums
        outs.append(np.ascontiguousarray((oT[:HS] / oT[HS : HS + 1]).T))
    return np.stack(outs, axis=0).astype(np.float32)
